# revision 55
# baseline (speedup 1.0000x reference)
"""Trainium2 Bass kernel for nn_Attention2 (dense transformer block with
softmax over the heads axis).

Computation per (n, t) batch b (B = n*t = 4096 total, X_b = x[n,:,t,:].T is
[vv=25, c=512]):
    qkv = X_b @ w_qkv.T, split into q,k,v heads [h=8, 25, hd=64]
    s[h,i,j] = (q[h,i,:] . k[h,j,:]) / 8      (scale folded into w_q on host)
    p = softmax over h (axis 0)
    o[h,i,:] = sum_j p[h,i,j] v[h,j,:]  -> [25, 512] -> @ w_proj.T
    out[n,:,t,:] = result.T
Sharding: data-parallel over n, 2 n-values (512 batches) per core, 8 cores.

v3 design (~576us vs v2's ~595us; PE busy ~93%, most overhead is the
irreducible fill floor of the small s/o matmuls + fixed ~7us context init;
tail iterations run proj BEFORE the o-section so it covers the last
group's softmax-chain latency; the v/o loop runs each o-bundle BEFORE the
v-chain so the PE does useful work while the first v-chain's PSUM WAR
partner -- the m7 qk evacuation -- clears the ACT queue):
 - x loaded ONCE per (nn,kc) into a contiguous [128, 6400] SBUF tile via
   4 chunked DMAs (3.2KB bursts; v2 loaded x TWICE in 50B bursts).  qk
   moving operands are contiguous slices of it; the 32-padded v-GEMM
   stationary tiles are produced on-chip by Pool-engine copies interleaved
   one-per-sub so they never block the latency-critical p2 multiplies
   (Pool FIFO is strict).  A matmul operand AP must optimize to ONE free
   dim, so zero-copy overlapping-window views are not possible.
 - DMA-issue costs ~0.7-1us PER INSTRUCTION on the issuing engine; the
   prologue splits issue across both HWDGE engines (SP + ACT) so group 0's
   weights+x land ~4us earlier, and ~28 dependency-free warmup matmuls
   bridge the prologue so the HAM clock gate reaches K=8/8 (2.4GHz) early.
 - per-group attention PSUM tiles hold all 4 subs ([128,4,4,25] = 1600B,
   one bank each; 2 psm + 4 po + 2 big = 8 banks): the across-group WAR is
   pre-satisfied a full iteration ahead, so the PE never waits on ACT/DVE
   evacuation.  (A persistent-tile + sub-parity variant raced: intermittent
   NaN via exp reading unwritten psm -> inf -> rD=0 -> inf*0.)
 - v_sb evacuation all-DVE, qk/po evacuation split ACT/DVE: ACT was the
   critical engine (its late evacs showed as ~350ns PE waits each group).
 - s-matmul emission runs par (PE row group) innermost so consecutive
   instructions target different row groups (LDW pull-ahead overlap).
 - 3-stage software pipeline per iteration as v2: qk+v GEMMs of group g,
   attention of g-1, proj of g-2, with s/o bundles interleaved between
   big-GEMM chunks (the small matmuls fill the chunks' evacuation-wait
   bubbles; batching by tiling mode measured WORSE).
 - everything f16 (fp8 fails the 2e-2 gate: ~4-7% rel err; f16 ~7e-4).
"""
import os
import numpy as np
import concourse.bass as bass
import concourse.mybir as mybir
import concourse.tile as tile
from concourse.bass_utils import run_bass_kernel_spmd
from concourse.vector_clock import ScopedClock, VectorClock

F32 = mybir.dt.float32
F16 = mybir.dt.float16

N_CORES = 8
NN_PER_CORE = 2        # n values per core
T = 256
VV = 25
C = 512
H = 8
HD = 64
TG = 16                # t values (batches) per group
NG = NN_PER_CORE * (T // TG)   # 32 groups per core
NGRUN = NG
NB = TG * VV           # 400 moving columns per group
XCOLS = T * VV         # 6400 columns of an xall tile


def _split_drain_and_barrier(self, tick_clock, wait_clock):
    # walrus caps sync-wait commands at 1 for CTRL_NO; split the kernel-tail
    # drain into one drain per pending proc.
    vc = tick_clock.global_clock
    n = len(vc)
    for i in range(n):
        if vc[i] == 0:
            continue
        sub = VectorClock([vc[j] if j == i else 0 for j in range(n)])
        d = self.nc.sync.drain()
        wait_clock.add_sem_waits(d.ins, ScopedClock({None: sub}))
    self.nc.all_engine_barrier()
    assert self.sems is not None
    popped = self.nc._tile_sem_poison_stack.pop()
    assert popped is self._sem_poison
    self.nc.clear_and_free_semaphores(list(self.sems.allocated().values()))
    self.nc.all_engine_barrier()


tile.TileContext._drain_and_barrier = _split_drain_and_barrier


def split_excess_waits(nc, limit=1):
    """walrus codegen allows very few sync-wait commands per instruction
    (1 for matmul/drain/DMA structs).  Move excess waits onto same-engine
    NoOp carriers inserted just before the instruction — same semantics,
    since each engine executes its queue in order."""
    k = 0
    for fn in nc.m.functions:
        for bb in fn.blocks:
            out = []
            for ins in bb.instructions:
                si = ins.sync_info
                waits = list(si.on_wait) if si is not None and si.on_wait else []
                if len(waits) > limit:
                    keep = waits[-limit:]
                    for w in waits[:-limit]:
                        nop = mybir.InstNoOp(
                            name=f"WC-{k}", ins=[], outs=[], engine=ins.engine
                        )
                        k += 1
                        nop.sync_info = mybir.SyncInfo(on_wait=[w], on_update=[])
                        out.append(nop)
                    si.on_wait = keep
                out.append(ins)
            bb.instructions[:] = out
    return k


def build_nc():

    nc = bass.Bass()
    X = nc.declare_dram_parameter("x", [NN_PER_CORE, C, T, VV], F16, isOutput=False)
    WQK = nc.declare_dram_parameter("wqkT", [C, 2 * C], F16, isOutput=False)
    WV = nc.declare_dram_parameter("wvT", [C, C], F16, isOutput=False)
    WP = nc.declare_dram_parameter("wprojT", [C, C], F16, isOutput=False)
    Y = nc.declare_dram_parameter("y", [NN_PER_CORE, C, T, VV], F16, isOutput=True)

    with tile.TileContext(nc) as tc:
        with (
            tc.tile_pool(name="consts", bufs=1) as consts,
            tc.tile_pool(name="xpool", bufs=2) as xpool,
            tc.tile_pool(name="qkpool", bufs=2) as qkpool,
            tc.tile_pool(name="vpool", bufs=2) as vpool,
            tc.tile_pool(name="softpool", bufs=2) as softpool,
            tc.tile_pool(name="otpool", bufs=3) as otpool,
            tc.tile_pool(name="finpool", bufs=4) as finpool,
            tc.tile_pool(name="pbig", bufs=2, space="PSUM") as pbig,
            tc.tile_pool(name="pattn", bufs=1, space="PSUM") as pattn,
        ):
            # ---- weights + x loads.  DMA-issue on the SP engine costs
            # ~0.7-1us PER INSTRUCTION, so the prologue splits issue across
            # the two HWDGE engines (SP + ACT) and uses few, large chunks. --
            NCH = 4
            CHT = T // NCH            # 64 t-values per chunk
            xall = [[None] * 4 for _ in range(NN_PER_CORE)]
            for nn in range(NN_PER_CORE):
                for kc in range(4):
                    xt = consts.tile([128, XCOLS], F16,
                                     tag=f"xall{nn}_{kc}", name=f"xall{nn}_{kc}")
                    xall[nn][kc] = xt

            def x_chunk_dma(nn, ch, eng):
                for kc in range(4):
                    eng.dma_start(
                        out=xall[nn][kc][:, ch * CHT * VV:(ch + 1) * CHT * VV]
                            .rearrange("p (t v) -> p t v", t=CHT),
                        in_=X[nn, kc * 128:(kc + 1) * 128,
                              ch * CHT:(ch + 1) * CHT, :],
                    )

            # ACT-DGE: first x chunk (gates the first matmul), then wp + ch1
            wqk_r, wv_r, wp_r = [], [], []
            x_chunk_dma(0, 0, nc.scalar)
            for kc in range(4):
                r2 = consts.tile([128, C], F16, tag=f"wpr{kc}", name=f"wpr{kc}")
                nc.scalar.dma_start(out=r2, in_=WP[kc * 128:(kc + 1) * 128, :])
                wp_r.append(r2)
            x_chunk_dma(0, 1, nc.scalar)
            # SP-DGE: qk weights (gate the first LDW), v weights, rest of x
            for kc in range(4):
                r0 = consts.tile([128, 2 * C], F16, tag=f"wqkr{kc}", name=f"wqkr{kc}")
                nc.sync.dma_start(out=r0, in_=WQK[kc * 128:(kc + 1) * 128, :])
                wqk_r.append(r0)
            for kc in range(4):
                r1 = consts.tile([128, C], F16, tag=f"wvr{kc}", name=f"wvr{kc}")
                nc.sync.dma_start(out=r1, in_=WV[kc * 128:(kc + 1) * 128, :])
                wv_r.append(r1)
            for ch in range(2, NCH):
                x_chunk_dma(0, ch, nc.sync)
            for ch in range(NCH):
                x_chunk_dma(1, ch, nc.sync)

            def qk_moving(g, kc):
                nn = g // (T // TG)
                t0 = (g % (T // TG)) * TG
                return xall[nn][kc][:, t0 * VV:t0 * VV + NB]

            # 32-padded x copies for the v-GEMM stationary, produced on-chip
            # from xall (Pool engine).  Emission is interleaved one copy per
            # attention sub so the latency-critical p2 multiplies never queue
            # behind them in Pool's strict FIFO.
            def pad_copy(g, kc):
                nn = g // (T // TG)
                t0 = (g % (T // TG)) * TG
                xq = xpool.tile([128, TG, 32], F16, tag=f"xp{kc}", name=f"xp{kc}")
                nc.gpsimd.tensor_copy(
                    xq[:, :, 0:VV],
                    xall[nn][kc][:, t0 * VV:t0 * VV + NB]
                        .rearrange("p (t v) -> p t v", t=TG),
                )
                return xq



            # ---- PE warmup: dependency-free dummy matmuls bridge the
            # prologue DMA wait so the HAM clock gate reaches K=8/8 (2.4GHz)
            # before the first real GEMM, instead of ~30us in. ----
            wsrc = consts.tile([128, 256], F16, tag="wsrc", name="wsrc")
            nc.gpsimd.memset(wsrc[:], 0.0)
            for _ in range(28):
                pwarm = pbig.tile([128, 200], F32, tag="big", name="pwarm")
                nc.tensor.matmul(
                    pwarm[:], wsrc[:, 0:128], wsrc[:, 0:200],
                    start=True, stop=True,
                )

            xp_of = {}       # g -> 4 xp tiles ([128, TG, 32] f16)
            qkT_of = {}      # g -> 8 qkT tiles ([128, NB] f16)
            vsb_of = {}      # g -> 4 v_sb tiles ([128, C] f16)
            p2_of = {}       # g -> 4 p2 tiles ([128, 4, 2, VV] f16)
            oT_of = {}       # g -> oT tile ([128, 4, NB] f16)

            xp_of[0] = [pad_copy(0, kc) for kc in range(4)]

            for it in range(NGRUN + 2):
                g1 = it          # GEMM stage
                g0 = it - 1      # attention stage
                gp = it - 2      # proj stage

                def emit_proj(gp):
                    nn = gp // (T // TG)
                    t0 = (gp % (T // TG)) * TG
                    oTp = oT_of.pop(gp)
                    for co in range(4):
                        pf = pbig.tile([128, NB], F32, tag="big", name="pf")
                        for kc in range(4):
                            nc.tensor.matmul(
                                pf[:],
                                wp_r[kc][:, co * 128:(co + 1) * 128],
                                oTp[:, kc, :],
                                start=(kc == 0), stop=(kc == 3),
                            )
                        fin = finpool.tile([128, NB], F16, tag="fin", name="fin")
                        nc.scalar.activation(
                            fin[:], pf[:], mybir.ActivationFunctionType.Copy,
                        )
                        nc.sync.dma_start(
                            out=Y[nn, co * 128:(co + 1) * 128, t0:t0 + TG, :],
                            in_=fin[:].rearrange("p (t v) -> p t v", t=TG),
                        )
                    # drop references for dead groups
                    xp_of.pop(gp, None)
                    qkT_of.pop(gp, None)
                    vsb_of.pop(gp, None)
                    p2_of.pop(gp, None)

                # ---------- attention psm (g0) interleaved with qk GEMM (g1) ----
                att = 0 <= g0 < NGRUN
                if att:
                    qkT = qkT_of[g0]
                    p2_l = []
                    # per-group attention PSUM tiles holding all 4 subs; the
                    # across-group WAR lands a full iteration later, so it is
                    # pre-satisfied when these matmuls issue
                    psm = [
                        pattn.tile([128, 4, 4, VV], F32, tag=f"psm{par}", name=f"psm{par}")
                        for par in range(2)
                    ]
                    po = [
                        pattn.tile([128, 4, 4, VV], F32, tag=f"po{b4}", name=f"po{b4}")
                        for b4 in range(4)
                    ]

                if g1 < NGRUN:
                    qkT_new = [
                        qkpool.tile([128, NB], F16, tag=f"qkT{m}", name=f"qkT{m}")
                        for m in range(8)
                    ]
                    qkT_of[g1] = qkT_new

                # interleave: psm bundles for sub s, then 2 qk m-chunks
                xp_next = []
                for step in range(4):
                    if att:
                        sub = step
                        # s-matmul bundles; par innermost alternates PE row
                        # groups so LDW pull-ahead can overlap
                        for m in range(4):
                            for b4 in range(4):
                                slot = 4 * sub + b4
                                for par in range(2):
                                    nc.tensor.matmul(
                                        psm[par][32 * b4:32 * b4 + VV, sub, m, :],
                                        qkT[4 + m][64 * par:64 * par + 64,
                                                   slot * VV:(slot + 1) * VV],
                                        qkT[m][64 * par:64 * par + 64,
                                               slot * VV:(slot + 1) * VV],
                                        start=True, stop=True,
                                        tile_position=(64 * par, 32 * b4),
                                    )
                        # softmax for this sub (ACT exp, DVE reduce/recip,
                        # Pool multiply)
                        e_t = softpool.tile([128, 2, 4, VV], F32, tag="e_t", name="e_t", bufs=4)
                        for par in range(2):
                            nc.scalar.activation(
                                e_t[:, par, :, :], psm[par][:, sub, :, :],
                                mybir.ActivationFunctionType.Exp,
                            )
                        D = softpool.tile([128, VV], F32, tag="D", name="D")
                        nc.vector.reduce_sum(
                            out=D[:],
                            in_=e_t[:].rearrange("p a m i -> p i (a m)"),
                            axis=mybir.AxisListType.X,
                        )
                        rD = softpool.tile([128, VV], F32, tag="rD", name="rD")
                        nc.vector.reciprocal(rD[:], D[:])
                        # p2 laid out [part, mp, e, i] (contiguous per head)
                        p2 = softpool.tile([128, 4, 2, VV], F16, tag="p2", name="p2", bufs=4)
                        nc.gpsimd.tensor_mul(
                            p2[:],
                            e_t[:].rearrange("p a m i -> p m a i"),
                            rD[:].unsqueeze(1).unsqueeze(1)
                                .broadcast_to([128, 4, 2, VV]),
                        )
                        p2_l.append(p2)
                        if g1 + 1 < NGRUN:
                            xp_next.append(pad_copy(g1 + 1, step))

                    if g1 < NGRUN:
                        for m in (2 * step, 2 * step + 1):
                            pq = pbig.tile([128, NB], F32, tag="big", name="pq")
                            for kc in range(4):
                                nc.tensor.matmul(
                                    pq[:],
                                    wqk_r[kc][:, m * 128:(m + 1) * 128],
                                    qk_moving(g1, kc),
                                    start=(kc == 0), stop=(kc == 3),
                                )
                            # contiguous evacuation; alternate engines
                            dst = qkT_new[m][:]
                            src = pq[:]
                            if m % 2 == 0:
                                nc.vector.tensor_copy(dst, src)
                            else:
                                nc.scalar.activation(
                                    dst, src, mybir.ActivationFunctionType.Copy,
                                )

                if att:
                    p2_of[g0] = p2_l
                if g1 + 1 < NGRUN:
                    while len(xp_next) < 4:
                        xp_next.append(pad_copy(g1 + 1, len(xp_next)))
                    xp_of[g1 + 1] = xp_next

                # tail iterations have no GEMM stage: run proj early so it
                # covers the last group's softmax-chain latency
                if g1 >= NGRUN and 0 <= gp < NGRUN:
                    emit_proj(gp)

                # ---------- v GEMM (g1) interleaved with po bundles (g0) ------
                if att:
                    oT = otpool.tile([128, 4, NB], F16, tag="oT", name="oT")
                    oT_of[g0] = oT
                for sub in range(4):
                    if att:
                        v_sb0 = vsb_of[g0][sub]
                        p2 = p2_of[g0][sub]
                        for mp in range(4):
                            for e in range(2):
                                for b4 in range(4):
                                    nc.tensor.matmul(
                                        po[b4][64 * e:64 * e + 64, sub, mp, :],
                                        v_sb0[32 * b4:32 * b4 + VV,
                                              128 * mp + 64 * e:128 * mp + 64 * e + 64],
                                        p2[32 * b4:32 * b4 + VV, mp, e, :],
                                        start=True, stop=True,
                                        tile_position=(32 * b4, 64 * e),
                                    )
                        # evacuate po -> oT slots (split ACT/DVE)
                        oT0 = oT_of[g0]
                        oT0v = oT0[:].rearrange("p m (s v) -> p m s v", v=VV)
                        for b4 in range(4):
                            if b4 % 2 == 0:
                                nc.vector.tensor_copy(
                                    oT0v[:, :, 4 * sub + b4, :], po[b4][:, sub, :, :],
                                )
                            else:
                                nc.scalar.activation(
                                    oT0v[:, :, 4 * sub + b4, :], po[b4][:, sub, :, :],
                                    mybir.ActivationFunctionType.Copy,
                                )

                    if g1 < NGRUN:
                        xp = xp_of[g1]
                        pv = pbig.tile([128, C], F32, tag="big", name="pv")
                        for kc in range(4):
                            nc.tensor.matmul(
                                pv[:],
                                xp[kc][:, 4 * sub:4 * sub + 4, :],
                                wv_r[kc][:],
                                start=(kc == 0), stop=(kc == 3),
                            )
                        v_sb = vpool.tile([128, C], F16, tag=f"v{sub}", name=f"v{sub}")
                        nc.vector.tensor_copy(v_sb[:], pv[:])
                        vsb_of.setdefault(g1, []).append(v_sb)

                # ---------- proj (gp) ----------------------------------------
                if g1 < NGRUN and 0 <= gp < NGRUN:
                    emit_proj(gp)

    return nc


LAST_RESULT = {}


def kernel(x: np.ndarray, w_qkv: np.ndarray, w_proj: np.ndarray,
           _trace: bool = False) -> np.ndarray:
    n, c, t, vv = x.shape
    assert (n, c, t, vv) == (16, 512, 256, 25)
    scale = np.float32((c // H) ** -0.5)

    wq = w_qkv[:c] * scale
    wk = w_qkv[c:2 * c]
    wv = w_qkv[2 * c:]
    wqkT = np.ascontiguousarray(np.concatenate([wq, wk], axis=0).T.astype(np.float16))
    wvT = np.ascontiguousarray(wv.T.astype(np.float16))
    wprojT = np.ascontiguousarray(w_proj.T.astype(np.float16))

    nc = build_nc()
    split_excess_waits(nc)
    in_maps = []
    for core in range(N_CORES):
        shard = np.ascontiguousarray(
            x[core * NN_PER_CORE:(core + 1) * NN_PER_CORE].astype(np.float16)
        )
        in_maps.append({"x": shard, "wqkT": wqkT, "wvT": wvT, "wprojT": wprojT})

    kw = {}
    if _trace:
        import tempfile
        kw = dict(trace=True, tmpdir=tempfile.mkdtemp(prefix="attn2_trace_"))
    res = run_bass_kernel_spmd(nc, in_maps, list(range(N_CORES)), **kw)
    LAST_RESULT["res"] = res
    LAST_RESULT["tmpdir"] = kw.get("tmpdir")
    out = np.empty((n, c, t, vv), dtype=np.float32)
    for core in range(N_CORES):
        out[core * NN_PER_CORE:(core + 1) * NN_PER_CORE] = \
            res.results[core]["y"].astype(np.float32)
    return out


# revision 60
# speedup vs baseline: 1.0019x; 1.0019x over previous
"""Trainium2 Bass kernel for nn_Attention2 (dense transformer block with
softmax over the heads axis).

Computation per (n, t) batch b (B = n*t = 4096 total, X_b = x[n,:,t,:].T is
[vv=25, c=512]):
    qkv = X_b @ w_qkv.T, split into q,k,v heads [h=8, 25, hd=64]
    s[h,i,j] = (q[h,i,:] . k[h,j,:]) / 8      (scale folded into w_q on host)
    p = softmax over h (axis 0)
    o[h,i,:] = sum_j p[h,i,j] v[h,j,:]  -> [25, 512] -> @ w_proj.T
    out[n,:,t,:] = result.T
Sharding: data-parallel over n, 2 n-values (512 batches) per core, 8 cores.

v3 design (~576us vs v2's ~595us; PE busy ~93%, most overhead is the
irreducible fill floor of the small s/o matmuls + fixed ~7us context init;
tail iterations run proj BEFORE the o-section so it covers the last
group's softmax-chain latency; the v/o loop runs each o-bundle BEFORE the
v-chain so the PE does useful work while the first v-chain's PSUM WAR
partner -- the m7 qk evacuation -- clears the ACT queue):
 - x loaded ONCE per (nn,kc) into a contiguous [128, 6400] SBUF tile via
   4 chunked DMAs (3.2KB bursts; v2 loaded x TWICE in 50B bursts).  qk
   moving operands are contiguous slices of it; the 32-padded v-GEMM
   stationary tiles are produced on-chip by Pool-engine copies interleaved
   one-per-sub so they never block the latency-critical p2 multiplies
   (Pool FIFO is strict).  A matmul operand AP must optimize to ONE free
   dim, so zero-copy overlapping-window views are not possible.
 - DMA-issue costs ~0.7-1us PER INSTRUCTION on the issuing engine; the
   prologue splits issue across both HWDGE engines (SP + ACT) so group 0's
   weights+x land ~4us earlier, and ~28 dependency-free warmup matmuls
   bridge the prologue so the HAM clock gate reaches K=8/8 (2.4GHz) early.
 - per-group attention PSUM tiles hold all 4 subs ([128,4,4,25] = 1600B,
   one bank each; 2 psm + 4 po + 2 big = 8 banks): the across-group WAR is
   pre-satisfied a full iteration ahead, so the PE never waits on ACT/DVE
   evacuation.  (A persistent-tile + sub-parity variant raced: intermittent
   NaN via exp reading unwritten psm -> inf -> rD=0 -> inf*0.)
 - v_sb evacuation all-DVE, qk/po evacuation split ACT/DVE: ACT was the
   critical engine (its late evacs showed as ~350ns PE waits each group).
 - s-matmul emission runs par (PE row group) innermost so consecutive
   instructions target different row groups (LDW pull-ahead overlap).
 - 3-stage software pipeline per iteration as v2: qk+v GEMMs of group g,
   attention of g-1, proj of g-2, with s/o bundles interleaved between
   big-GEMM chunks (the small matmuls fill the chunks' evacuation-wait
   bubbles; batching by tiling mode measured WORSE).
 - everything f16 (fp8 fails the 2e-2 gate: ~4-7% rel err; f16 ~7e-4).
"""
import os
import numpy as np
import concourse.bass as bass
import concourse.mybir as mybir
import concourse.tile as tile
from concourse.bass_utils import run_bass_kernel_spmd
from concourse.vector_clock import ScopedClock, VectorClock

F32 = mybir.dt.float32
F16 = mybir.dt.float16

N_CORES = 8
NN_PER_CORE = 2        # n values per core
T = 256
VV = 25
C = 512
H = 8
HD = 64
TG = 16                # t values (batches) per group
NG = NN_PER_CORE * (T // TG)   # 32 groups per core
NGRUN = NG
NB = TG * VV           # 400 moving columns per group
XCOLS = T * VV         # 6400 columns of an xall tile


def _split_drain_and_barrier(self, tick_clock, wait_clock):
    # walrus caps sync-wait commands at 1 for CTRL_NO; split the kernel-tail
    # drain into one drain per pending proc.
    vc = tick_clock.global_clock
    n = len(vc)
    for i in range(n):
        if vc[i] == 0:
            continue
        sub = VectorClock([vc[j] if j == i else 0 for j in range(n)])
        d = self.nc.sync.drain()
        wait_clock.add_sem_waits(d.ins, ScopedClock({None: sub}))
    self.nc.all_engine_barrier()
    assert self.sems is not None
    popped = self.nc._tile_sem_poison_stack.pop()
    assert popped is self._sem_poison
    self.nc.clear_and_free_semaphores(list(self.sems.allocated().values()))
    self.nc.all_engine_barrier()


tile.TileContext._drain_and_barrier = _split_drain_and_barrier


def split_excess_waits(nc, limit=1):
    """walrus codegen allows very few sync-wait commands per instruction
    (1 for matmul/drain/DMA structs).  Move excess waits onto same-engine
    NoOp carriers inserted just before the instruction — same semantics,
    since each engine executes its queue in order."""
    k = 0
    for fn in nc.m.functions:
        for bb in fn.blocks:
            out = []
            for ins in bb.instructions:
                si = ins.sync_info
                waits = list(si.on_wait) if si is not None and si.on_wait else []
                if len(waits) > limit:
                    keep = waits[-limit:]
                    for w in waits[:-limit]:
                        nop = mybir.InstNoOp(
                            name=f"WC-{k}", ins=[], outs=[], engine=ins.engine
                        )
                        k += 1
                        nop.sync_info = mybir.SyncInfo(on_wait=[w], on_update=[])
                        out.append(nop)
                    si.on_wait = keep
                out.append(ins)
            bb.instructions[:] = out
    return k


def build_nc():

    nc = bass.Bass()
    X = nc.declare_dram_parameter("x", [NN_PER_CORE, C, T, VV], F16, isOutput=False)
    WQK = nc.declare_dram_parameter("wqkT", [C, 2 * C], F16, isOutput=False)
    WV = nc.declare_dram_parameter("wvT", [C, C], F16, isOutput=False)
    WP = nc.declare_dram_parameter("wprojT", [C, C], F16, isOutput=False)
    Y = nc.declare_dram_parameter("y", [NN_PER_CORE, C, T, VV], F16, isOutput=True)

    with tile.TileContext(nc) as tc:
        with (
            tc.tile_pool(name="consts", bufs=1) as consts,
            tc.tile_pool(name="xpool", bufs=2) as xpool,
            tc.tile_pool(name="qkpool", bufs=2) as qkpool,
            tc.tile_pool(name="vpool", bufs=2) as vpool,
            tc.tile_pool(name="softpool", bufs=2) as softpool,
            tc.tile_pool(name="otpool", bufs=3) as otpool,
            tc.tile_pool(name="finpool", bufs=4) as finpool,
            tc.tile_pool(name="pbig", bufs=2, space="PSUM") as pbig,
            tc.tile_pool(name="pattn", bufs=1, space="PSUM") as pattn,
        ):
            # ---- weights + x loads.  DMA-issue on the SP engine costs
            # ~0.7-1us PER INSTRUCTION, so the prologue splits issue across
            # the two HWDGE engines (SP + ACT) and uses few, large chunks. --
            NCH = 4
            CHT = T // NCH            # 64 t-values per chunk
            xall = [[None] * 4 for _ in range(NN_PER_CORE)]
            for nn in range(NN_PER_CORE):
                for kc in range(4):
                    xt = consts.tile([128, XCOLS], F16,
                                     tag=f"xall{nn}_{kc}", name=f"xall{nn}_{kc}")
                    xall[nn][kc] = xt

            def x_chunk_dma(nn, ch, eng):
                for kc in range(4):
                    eng.dma_start(
                        out=xall[nn][kc][:, ch * CHT * VV:(ch + 1) * CHT * VV]
                            .rearrange("p (t v) -> p t v", t=CHT),
                        in_=X[nn, kc * 128:(kc + 1) * 128,
                              ch * CHT:(ch + 1) * CHT, :],
                    )

            # ACT-DGE: first x chunk (gates the first matmul), then wp + ch1
            wqk_r, wv_r, wp_r = [], [], []
            x_chunk_dma(0, 0, nc.scalar)
            for kc in range(4):
                r2 = consts.tile([128, C], F16, tag=f"wpr{kc}", name=f"wpr{kc}")
                nc.scalar.dma_start(out=r2, in_=WP[kc * 128:(kc + 1) * 128, :])
                wp_r.append(r2)
            x_chunk_dma(0, 1, nc.scalar)
            # SP-DGE: qk weights (gate the first LDW), v weights, rest of x
            for kc in range(4):
                r0 = consts.tile([128, 2 * C], F16, tag=f"wqkr{kc}", name=f"wqkr{kc}")
                nc.sync.dma_start(out=r0, in_=WQK[kc * 128:(kc + 1) * 128, :])
                wqk_r.append(r0)
            for kc in range(4):
                r1 = consts.tile([128, C], F16, tag=f"wvr{kc}", name=f"wvr{kc}")
                nc.sync.dma_start(out=r1, in_=WV[kc * 128:(kc + 1) * 128, :])
                wv_r.append(r1)
            for ch in range(2, NCH):
                x_chunk_dma(0, ch, nc.sync)
            for ch in range(NCH):
                x_chunk_dma(1, ch, nc.sync)

            def qk_moving(g, kc):
                nn = g // (T // TG)
                t0 = (g % (T // TG)) * TG
                return xall[nn][kc][:, t0 * VV:t0 * VV + NB]

            # 32-padded x copies for the v-GEMM stationary, produced on-chip
            # from xall (Pool engine).  Emission is interleaved one copy per
            # attention sub so the latency-critical p2 multiplies never queue
            # behind them in Pool's strict FIFO.
            def pad_copy(g, kc):
                nn = g // (T // TG)
                t0 = (g % (T // TG)) * TG
                xq = xpool.tile([128, TG, 32], F16, tag=f"xp{kc}", name=f"xp{kc}")
                nc.gpsimd.tensor_copy(
                    xq[:, :, 0:VV],
                    xall[nn][kc][:, t0 * VV:t0 * VV + NB]
                        .rearrange("p (t v) -> p t v", t=TG),
                )
                return xq



            # ---- PE warmup: dependency-free dummy matmuls bridge the
            # prologue DMA wait so the HAM clock gate reaches K=8/8 (2.4GHz)
            # before the first real GEMM, instead of ~30us in. ----
            wsrc = consts.tile([128, 256], F16, tag="wsrc", name="wsrc")
            nc.gpsimd.memset(wsrc[:], 0.0)
            for _ in range(28):
                pwarm = pbig.tile([128, 200], F32, tag="big", name="pwarm")
                nc.tensor.matmul(
                    pwarm[:], wsrc[:, 0:128], wsrc[:, 0:200],
                    start=True, stop=True,
                )

            xp_of = {}       # g -> 4 xp tiles ([128, TG, 32] f16)
            qkT_of = {}      # g -> 8 qkT tiles ([128, NB] f16)
            vsb_of = {}      # g -> 4 v_sb tiles ([128, C] f16)
            p2_of = {}       # g -> 4 p2 tiles ([128, 4, 2, VV] f16)
            oT_of = {}       # g -> oT tile ([128, 4, NB] f16)

            xp_of[0] = [pad_copy(0, kc) for kc in range(4)]

            for it in range(NGRUN + 2):
                g1 = it          # GEMM stage
                g0 = it - 1      # attention stage
                gp = it - 2      # proj stage

                def emit_proj(gp):
                    nn = gp // (T // TG)
                    t0 = (gp % (T // TG)) * TG
                    oTp = oT_of.pop(gp)
                    for co in range(4):
                        pf = pbig.tile([128, NB], F32, tag="big", name="pf")
                        for kc in range(4):
                            nc.tensor.matmul(
                                pf[:],
                                wp_r[kc][:, co * 128:(co + 1) * 128],
                                oTp[:, kc, :],
                                start=(kc == 0), stop=(kc == 3),
                            )
                        fin = finpool.tile([128, NB], F16, tag="fin", name="fin")
                        nc.scalar.activation(
                            fin[:], pf[:], mybir.ActivationFunctionType.Copy,
                        )
                        nc.sync.dma_start(
                            out=Y[nn, co * 128:(co + 1) * 128, t0:t0 + TG, :],
                            in_=fin[:].rearrange("p (t v) -> p t v", t=TG),
                        )
                    # drop references for dead groups
                    xp_of.pop(gp, None)
                    qkT_of.pop(gp, None)
                    vsb_of.pop(gp, None)
                    p2_of.pop(gp, None)

                # ---------- attention psm (g0) interleaved with qk GEMM (g1) ----
                att = 0 <= g0 < NGRUN
                if att:
                    qkT = qkT_of[g0]
                    p2_l = []
                    # per-group attention PSUM tiles holding all 4 subs; the
                    # across-group WAR lands a full iteration later, so it is
                    # pre-satisfied when these matmuls issue
                    psm = [
                        pattn.tile([128, 4, 4, VV], F32, tag=f"psm{par}", name=f"psm{par}")
                        for par in range(2)
                    ]
                    po = [
                        pattn.tile([128, 4, 4, VV], F32, tag=f"po{b4}", name=f"po{b4}")
                        for b4 in range(4)
                    ]

                if g1 < NGRUN:
                    qkT_new = [
                        qkpool.tile([128, NB], F16, tag=f"qkT{m}", name=f"qkT{m}")
                        for m in range(8)
                    ]
                    qkT_of[g1] = qkT_new

                # interleave: psm bundles for sub s, then 2 qk m-chunks
                xp_next = []
                for step in range(4):
                    if att:
                        sub = step
                        # s-matmul bundles; par innermost alternates PE row
                        # groups so LDW pull-ahead can overlap
                        for m in range(4):
                            for b4 in range(4):
                                slot = 4 * sub + b4
                                for par in range(2):
                                    nc.tensor.matmul(
                                        psm[par][32 * b4:32 * b4 + VV, sub, m, :],
                                        qkT[4 + m][64 * par:64 * par + 64,
                                                   slot * VV:(slot + 1) * VV],
                                        qkT[m][64 * par:64 * par + 64,
                                               slot * VV:(slot + 1) * VV],
                                        start=True, stop=True,
                                        tile_position=(64 * par, 32 * b4),
                                    )
                        # softmax for this sub (ACT exp, DVE reduce/recip,
                        # Pool multiply)
                        e_t = softpool.tile([128, 2, 4, VV], F16, tag="e_t", name="e_t", bufs=4)
                        for par in range(2):
                            nc.scalar.activation(
                                e_t[:, par, :, :], psm[par][:, sub, :, :],
                                mybir.ActivationFunctionType.Exp,
                            )
                        D = softpool.tile([128, VV], F32, tag="D", name="D")
                        with nc.allow_low_precision(
                            reason="softmax over 8 heads: f16 exp values "
                                   "<=e^5, sum rel err ~5e-4 vs 2e-2 gate"
                        ):
                            nc.vector.reduce_sum(
                                out=D[:],
                                in_=e_t[:].rearrange("p a m i -> p i (a m)"),
                                axis=mybir.AxisListType.X,
                            )
                        rD = softpool.tile([128, VV], F16, tag="rD", name="rD")
                        with nc.allow_low_precision(
                            reason="f16 1/D: rel err ~5e-4 vs 2e-2 gate"
                        ):
                            nc.vector.reciprocal(rD[:], D[:])
                        # p2 laid out [part, mp, e, i] (contiguous per head)
                        p2 = softpool.tile([128, 4, 2, VV], F16, tag="p2", name="p2", bufs=4)
                        nc.gpsimd.tensor_mul(
                            p2[:],
                            e_t[:].rearrange("p a m i -> p m a i"),
                            rD[:].unsqueeze(1).unsqueeze(1)
                                .broadcast_to([128, 4, 2, VV]),
                        )
                        p2_l.append(p2)
                        if g1 + 1 < NGRUN:
                            xp_next.append(pad_copy(g1 + 1, step))

                    if g1 < NGRUN:
                        for m in (2 * step, 2 * step + 1):
                            pq = pbig.tile([128, NB], F32, tag="big", name="pq")
                            for kc in range(4):
                                nc.tensor.matmul(
                                    pq[:],
                                    wqk_r[kc][:, m * 128:(m + 1) * 128],
                                    qk_moving(g1, kc),
                                    start=(kc == 0), stop=(kc == 3),
                                )
                            # contiguous evacuation; alternate engines
                            dst = qkT_new[m][:]
                            src = pq[:]
                            if m % 2 == 0:
                                nc.vector.tensor_copy(dst, src)
                            else:
                                nc.scalar.activation(
                                    dst, src, mybir.ActivationFunctionType.Copy,
                                )

                if att:
                    p2_of[g0] = p2_l
                if g1 + 1 < NGRUN:
                    while len(xp_next) < 4:
                        xp_next.append(pad_copy(g1 + 1, len(xp_next)))
                    xp_of[g1 + 1] = xp_next

                # tail iterations have no GEMM stage: run proj early so it
                # covers the last group's softmax-chain latency
                if g1 >= NGRUN and 0 <= gp < NGRUN:
                    emit_proj(gp)

                # ---------- v GEMM (g1) interleaved with po bundles (g0) ------
                if att:
                    oT = otpool.tile([128, 4, NB], F16, tag="oT", name="oT")
                    oT_of[g0] = oT
                for sub in range(4):
                    if att:
                        v_sb0 = vsb_of[g0][sub]
                        p2 = p2_of[g0][sub]
                        for mp in range(4):
                            for e in range(2):
                                for b4 in range(4):
                                    nc.tensor.matmul(
                                        po[b4][64 * e:64 * e + 64, sub, mp, :],
                                        v_sb0[32 * b4:32 * b4 + VV,
                                              128 * mp + 64 * e:128 * mp + 64 * e + 64],
                                        p2[32 * b4:32 * b4 + VV, mp, e, :],
                                        start=True, stop=True,
                                        tile_position=(32 * b4, 64 * e),
                                    )
                        # evacuate po -> oT slots (split ACT/DVE)
                        oT0 = oT_of[g0]
                        oT0v = oT0[:].rearrange("p m (s v) -> p m s v", v=VV)
                        for b4 in range(4):
                            if b4 % 2 == 0:
                                nc.vector.tensor_copy(
                                    oT0v[:, :, 4 * sub + b4, :], po[b4][:, sub, :, :],
                                )
                            else:
                                nc.scalar.activation(
                                    oT0v[:, :, 4 * sub + b4, :], po[b4][:, sub, :, :],
                                    mybir.ActivationFunctionType.Copy,
                                )

                    if g1 < NGRUN:
                        xp = xp_of[g1]
                        pv = pbig.tile([128, C], F32, tag="big", name="pv")
                        for kc in range(4):
                            nc.tensor.matmul(
                                pv[:],
                                xp[kc][:, 4 * sub:4 * sub + 4, :],
                                wv_r[kc][:],
                                start=(kc == 0), stop=(kc == 3),
                            )
                        v_sb = vpool.tile([128, C], F16, tag=f"v{sub}", name=f"v{sub}")
                        nc.vector.tensor_copy(v_sb[:], pv[:])
                        vsb_of.setdefault(g1, []).append(v_sb)

                # ---------- proj (gp) ----------------------------------------
                if g1 < NGRUN and 0 <= gp < NGRUN:
                    emit_proj(gp)

    return nc


LAST_RESULT = {}


def kernel(x: np.ndarray, w_qkv: np.ndarray, w_proj: np.ndarray,
           _trace: bool = False) -> np.ndarray:
    n, c, t, vv = x.shape
    assert (n, c, t, vv) == (16, 512, 256, 25)
    scale = np.float32((c // H) ** -0.5)

    wq = w_qkv[:c] * scale
    wk = w_qkv[c:2 * c]
    wv = w_qkv[2 * c:]
    wqkT = np.ascontiguousarray(np.concatenate([wq, wk], axis=0).T.astype(np.float16))
    wvT = np.ascontiguousarray(wv.T.astype(np.float16))
    wprojT = np.ascontiguousarray(w_proj.T.astype(np.float16))

    nc = build_nc()
    split_excess_waits(nc)
    in_maps = []
    for core in range(N_CORES):
        shard = np.ascontiguousarray(
            x[core * NN_PER_CORE:(core + 1) * NN_PER_CORE].astype(np.float16)
        )
        in_maps.append({"x": shard, "wqkT": wqkT, "wvT": wvT, "wprojT": wprojT})

    kw = {}
    if _trace:
        import tempfile
        kw = dict(trace=True, tmpdir=tempfile.mkdtemp(prefix="attn2_trace_"))
    res = run_bass_kernel_spmd(nc, in_maps, list(range(N_CORES)), **kw)
    LAST_RESULT["res"] = res
    LAST_RESULT["tmpdir"] = kw.get("tmpdir")
    out = np.empty((n, c, t, vv), dtype=np.float32)
    for core in range(N_CORES):
        out[core * NN_PER_CORE:(core + 1) * NN_PER_CORE] = \
            res.results[core]["y"].astype(np.float32)
    return out


# revision 61
# speedup vs baseline: 1.0028x; 1.0009x over previous
"""Trainium2 Bass kernel for nn_Attention2 (dense transformer block with
softmax over the heads axis).

Computation per (n, t) batch b (B = n*t = 4096 total, X_b = x[n,:,t,:].T is
[vv=25, c=512]):
    qkv = X_b @ w_qkv.T, split into q,k,v heads [h=8, 25, hd=64]
    s[h,i,j] = (q[h,i,:] . k[h,j,:]) / 8      (scale folded into w_q on host)
    p = softmax over h (axis 0)
    o[h,i,:] = sum_j p[h,i,j] v[h,j,:]  -> [25, 512] -> @ w_proj.T
    out[n,:,t,:] = result.T
Sharding: data-parallel over n, 2 n-values (512 batches) per core, 8 cores.

v3 design (~580us vs v2's ~595us; PE busy ~93%, most overhead is the
irreducible fill floor of the small s/o matmuls + fixed ~7us context init;
tail iterations run proj BEFORE the o-section so it covers the last
group's softmax-chain latency):
 - x loaded ONCE per (nn,kc) into a contiguous [128, 6400] SBUF tile via
   4 chunked DMAs (3.2KB bursts; v2 loaded x TWICE in 50B bursts).  qk
   moving operands are contiguous slices of it; the 32-padded v-GEMM
   stationary tiles are produced on-chip by Pool-engine copies interleaved
   one-per-sub so they never block the latency-critical p2 multiplies
   (Pool FIFO is strict).  A matmul operand AP must optimize to ONE free
   dim, so zero-copy overlapping-window views are not possible.
 - DMA-issue costs ~0.7-1us PER INSTRUCTION on the issuing engine; the
   prologue splits issue across both HWDGE engines (SP + ACT) so group 0's
   weights+x land ~4us earlier, and ~28 dependency-free warmup matmuls
   bridge the prologue so the HAM clock gate reaches K=8/8 (2.4GHz) early.
 - per-group attention PSUM tiles hold all 4 subs ([128,4,4,25] = 1600B,
   one bank each; 2 psm + 4 po + 2 big = 8 banks): the across-group WAR is
   pre-satisfied a full iteration ahead, so the PE never waits on ACT/DVE
   evacuation.  (A persistent-tile + sub-parity variant raced: intermittent
   NaN via exp reading unwritten psm -> inf -> rD=0 -> inf*0.)
 - v_sb evacuation all-DVE, qk/po evacuation split ACT/DVE: ACT was the
   critical engine (its late evacs showed as ~350ns PE waits each group).
 - s-matmul emission runs par (PE row group) innermost so consecutive
   instructions target different row groups (LDW pull-ahead overlap).
 - 3-stage software pipeline per iteration as v2: qk+v GEMMs of group g,
   attention of g-1, proj of g-2, with s/o bundles interleaved between
   big-GEMM chunks (the small matmuls fill the chunks' evacuation-wait
   bubbles; batching by tiling mode measured WORSE).
 - everything f16 (fp8 fails the 2e-2 gate: ~4-7% rel err; f16 ~7e-4).
"""
import os
import numpy as np
import concourse.bass as bass
import concourse.mybir as mybir
import concourse.tile as tile
from concourse.bass_utils import run_bass_kernel_spmd
from concourse.vector_clock import ScopedClock, VectorClock

F32 = mybir.dt.float32
F16 = mybir.dt.float16

N_CORES = 8
NN_PER_CORE = 2        # n values per core
T = 256
VV = 25
C = 512
H = 8
HD = 64
TG = 16                # t values (batches) per group
NG = NN_PER_CORE * (T // TG)   # 32 groups per core
NGRUN = NG
NB = TG * VV           # 400 moving columns per group
XCOLS = T * VV         # 6400 columns of an xall tile


def _split_drain_and_barrier(self, tick_clock, wait_clock):
    # walrus caps sync-wait commands at 1 for CTRL_NO; split the kernel-tail
    # drain into one drain per pending proc.
    vc = tick_clock.global_clock
    n = len(vc)
    for i in range(n):
        if vc[i] == 0:
            continue
        sub = VectorClock([vc[j] if j == i else 0 for j in range(n)])
        d = self.nc.sync.drain()
        wait_clock.add_sem_waits(d.ins, ScopedClock({None: sub}))
    self.nc.all_engine_barrier()
    assert self.sems is not None
    popped = self.nc._tile_sem_poison_stack.pop()
    assert popped is self._sem_poison
    self.nc.clear_and_free_semaphores(list(self.sems.allocated().values()))
    self.nc.all_engine_barrier()


tile.TileContext._drain_and_barrier = _split_drain_and_barrier


def split_excess_waits(nc, limit=1):
    """walrus codegen allows very few sync-wait commands per instruction
    (1 for matmul/drain/DMA structs).  Move excess waits onto same-engine
    NoOp carriers inserted just before the instruction — same semantics,
    since each engine executes its queue in order."""
    k = 0
    for fn in nc.m.functions:
        for bb in fn.blocks:
            out = []
            for ins in bb.instructions:
                si = ins.sync_info
                waits = list(si.on_wait) if si is not None and si.on_wait else []
                if len(waits) > limit:
                    keep = waits[-limit:]
                    for w in waits[:-limit]:
                        nop = mybir.InstNoOp(
                            name=f"WC-{k}", ins=[], outs=[], engine=ins.engine
                        )
                        k += 1
                        nop.sync_info = mybir.SyncInfo(on_wait=[w], on_update=[])
                        out.append(nop)
                    si.on_wait = keep
                out.append(ins)
            bb.instructions[:] = out
    return k


def build_nc():

    nc = bass.Bass()
    X = nc.declare_dram_parameter("x", [NN_PER_CORE, C, T, VV], F16, isOutput=False)
    WQK = nc.declare_dram_parameter("wqkT", [C, 2 * C], F16, isOutput=False)
    WV = nc.declare_dram_parameter("wvT", [C, C], F16, isOutput=False)
    WP = nc.declare_dram_parameter("wprojT", [C, C], F16, isOutput=False)
    Y = nc.declare_dram_parameter("y", [NN_PER_CORE, C, T, VV], F16, isOutput=True)

    with tile.TileContext(nc) as tc:
        with (
            tc.tile_pool(name="consts", bufs=1) as consts,
            tc.tile_pool(name="xpool", bufs=2) as xpool,
            tc.tile_pool(name="qkpool", bufs=2) as qkpool,
            tc.tile_pool(name="vpool", bufs=2) as vpool,
            tc.tile_pool(name="softpool", bufs=2) as softpool,
            tc.tile_pool(name="otpool", bufs=3) as otpool,
            tc.tile_pool(name="finpool", bufs=4) as finpool,
            tc.tile_pool(name="pbig", bufs=2, space="PSUM") as pbig,
            tc.tile_pool(name="pattn", bufs=1, space="PSUM") as pattn,
        ):
            # ---- weights + x loads.  DMA-issue on the SP engine costs
            # ~0.7-1us PER INSTRUCTION, so the prologue splits issue across
            # the two HWDGE engines (SP + ACT) and uses few, large chunks. --
            NCH = 4
            CHT = T // NCH            # 64 t-values per chunk
            xall = [[None] * 4 for _ in range(NN_PER_CORE)]
            for nn in range(NN_PER_CORE):
                for kc in range(4):
                    xt = consts.tile([128, XCOLS], F16,
                                     tag=f"xall{nn}_{kc}", name=f"xall{nn}_{kc}")
                    xall[nn][kc] = xt

            def x_chunk_dma(nn, ch, eng):
                for kc in range(4):
                    eng.dma_start(
                        out=xall[nn][kc][:, ch * CHT * VV:(ch + 1) * CHT * VV]
                            .rearrange("p (t v) -> p t v", t=CHT),
                        in_=X[nn, kc * 128:(kc + 1) * 128,
                              ch * CHT:(ch + 1) * CHT, :],
                    )

            # ACT-DGE: first x chunk (gates the first matmul), then wp + ch1
            wqk_r, wv_r, wp_r = [], [], []
            x_chunk_dma(0, 0, nc.scalar)
            for kc in range(4):
                r2 = consts.tile([128, C], F16, tag=f"wpr{kc}", name=f"wpr{kc}")
                nc.scalar.dma_start(out=r2, in_=WP[kc * 128:(kc + 1) * 128, :])
                wp_r.append(r2)
            x_chunk_dma(0, 1, nc.scalar)
            # SP-DGE: qk weights (gate the first LDW), v weights, rest of x
            for kc in range(4):
                r0 = consts.tile([128, 2 * C], F16, tag=f"wqkr{kc}", name=f"wqkr{kc}")
                nc.sync.dma_start(out=r0, in_=WQK[kc * 128:(kc + 1) * 128, :])
                wqk_r.append(r0)
            for kc in range(4):
                r1 = consts.tile([128, C], F16, tag=f"wvr{kc}", name=f"wvr{kc}")
                nc.sync.dma_start(out=r1, in_=WV[kc * 128:(kc + 1) * 128, :])
                wv_r.append(r1)
            for ch in range(2, NCH):
                x_chunk_dma(0, ch, nc.sync)
            for ch in range(NCH):
                x_chunk_dma(1, ch, nc.sync)

            def qk_moving(g, kc):
                nn = g // (T // TG)
                t0 = (g % (T // TG)) * TG
                return xall[nn][kc][:, t0 * VV:t0 * VV + NB]

            # 32-padded x copies for the v-GEMM stationary, produced on-chip
            # from xall (Pool engine).  Emission is interleaved one copy per
            # attention sub so the latency-critical p2 multiplies never queue
            # behind them in Pool's strict FIFO.
            def pad_copy(g, kc):
                nn = g // (T // TG)
                t0 = (g % (T // TG)) * TG
                xq = xpool.tile([128, TG, 32], F16, tag=f"xp{kc}", name=f"xp{kc}")
                nc.gpsimd.tensor_copy(
                    xq[:, :, 0:VV],
                    xall[nn][kc][:, t0 * VV:t0 * VV + NB]
                        .rearrange("p (t v) -> p t v", t=TG),
                )
                return xq



            # ---- PE warmup: dependency-free dummy matmuls bridge the
            # prologue DMA wait so the HAM clock gate reaches K=8/8 (2.4GHz)
            # before the first real GEMM, instead of ~30us in. ----
            wsrc = consts.tile([128, 256], F16, tag="wsrc", name="wsrc")
            nc.gpsimd.memset(wsrc[:], 0.0)
            for _ in range(28):
                pwarm = pbig.tile([128, 200], F32, tag="big", name="pwarm")
                nc.tensor.matmul(
                    pwarm[:], wsrc[:, 0:128], wsrc[:, 0:200],
                    start=True, stop=True,
                )

            xp_of = {}       # g -> 4 xp tiles ([128, TG, 32] f16)
            qkT_of = {}      # g -> 8 qkT tiles ([128, NB] f16)
            vsb_of = {}      # g -> 4 v_sb tiles ([128, C] f16)
            p2_of = {}       # g -> 4 p2 tiles ([128, 4, 2, VV] f16)
            oT_of = {}       # g -> oT tile ([128, 4, NB] f16)

            xp_of[0] = [pad_copy(0, kc) for kc in range(4)]

            for it in range(NGRUN + 2):
                g1 = it          # GEMM stage
                g0 = it - 1      # attention stage
                gp = it - 2      # proj stage

                def emit_proj(gp):
                    nn = gp // (T // TG)
                    t0 = (gp % (T // TG)) * TG
                    oTp = oT_of.pop(gp)
                    for co in range(4):
                        pf = pbig.tile([128, NB], F32, tag="big", name="pf")
                        for kc in range(4):
                            nc.tensor.matmul(
                                pf[:],
                                wp_r[kc][:, co * 128:(co + 1) * 128],
                                oTp[:, kc, :],
                                start=(kc == 0), stop=(kc == 3),
                            )
                        fin = finpool.tile([128, NB], F16, tag="fin", name="fin")
                        nc.scalar.activation(
                            fin[:], pf[:], mybir.ActivationFunctionType.Copy,
                        )
                        nc.sync.dma_start(
                            out=Y[nn, co * 128:(co + 1) * 128, t0:t0 + TG, :],
                            in_=fin[:].rearrange("p (t v) -> p t v", t=TG),
                        )
                    # drop references for dead groups
                    xp_of.pop(gp, None)
                    qkT_of.pop(gp, None)
                    vsb_of.pop(gp, None)
                    p2_of.pop(gp, None)

                # ---------- attention psm (g0) interleaved with qk GEMM (g1) ----
                att = 0 <= g0 < NGRUN
                if att:
                    qkT = qkT_of[g0]
                    p2_l = []
                    # per-group attention PSUM tiles holding all 4 subs; the
                    # across-group WAR lands a full iteration later, so it is
                    # pre-satisfied when these matmuls issue
                    psm = [
                        pattn.tile([128, 4, 4, VV], F32, tag=f"psm{par}", name=f"psm{par}")
                        for par in range(2)
                    ]
                    po = [
                        pattn.tile([128, 4, 4, VV], F32, tag=f"po{b4}", name=f"po{b4}")
                        for b4 in range(4)
                    ]

                if g1 < NGRUN:
                    qkT_new = [
                        qkpool.tile([128, NB], F16, tag=f"qkT{m}", name=f"qkT{m}")
                        for m in range(8)
                    ]
                    qkT_of[g1] = qkT_new

                # interleave: psm bundles for sub s, then 2 qk m-chunks
                xp_next = []
                for step in range(4):
                    if att:
                        sub = step
                        # s-matmul bundles; par innermost alternates PE row
                        # groups so LDW pull-ahead can overlap
                        for m in range(4):
                            for b4 in range(4):
                                slot = 4 * sub + b4
                                for par in range(2):
                                    nc.tensor.matmul(
                                        psm[par][32 * b4:32 * b4 + VV, sub, m, :],
                                        qkT[4 + m][64 * par:64 * par + 64,
                                                   slot * VV:(slot + 1) * VV],
                                        qkT[m][64 * par:64 * par + 64,
                                               slot * VV:(slot + 1) * VV],
                                        start=True, stop=True,
                                        tile_position=(64 * par, 32 * b4),
                                    )
                        # softmax for this sub (ACT exp, DVE reduce/recip,
                        # Pool multiply)
                        e_t = softpool.tile([128, 2, 4, VV], F32, tag="e_t", name="e_t", bufs=4)
                        for par in range(2):
                            nc.scalar.activation(
                                e_t[:, par, :, :], psm[par][:, sub, :, :],
                                mybir.ActivationFunctionType.Exp,
                            )
                        D = softpool.tile([128, VV], F32, tag="D", name="D")
                        nc.vector.reduce_sum(
                            out=D[:],
                            in_=e_t[:].rearrange("p a m i -> p i (a m)"),
                            axis=mybir.AxisListType.X,
                        )
                        rD = softpool.tile([128, VV], F32, tag="rD", name="rD")
                        nc.vector.reciprocal(rD[:], D[:])
                        # p2 laid out [part, mp, e, i] (contiguous per head)
                        p2 = softpool.tile([128, 4, 2, VV], F16, tag="p2", name="p2", bufs=4)
                        nc.gpsimd.tensor_mul(
                            p2[:],
                            e_t[:].rearrange("p a m i -> p m a i"),
                            rD[:].unsqueeze(1).unsqueeze(1)
                                .broadcast_to([128, 4, 2, VV]),
                        )
                        p2_l.append(p2)
                        if g1 + 1 < NGRUN:
                            xp_next.append(pad_copy(g1 + 1, step))

                    if g1 < NGRUN:
                        for m in (2 * step, 2 * step + 1):
                            pq = pbig.tile([128, NB], F32, tag="big", name="pq")
                            for kc in range(4):
                                nc.tensor.matmul(
                                    pq[:],
                                    wqk_r[kc][:, m * 128:(m + 1) * 128],
                                    qk_moving(g1, kc),
                                    start=(kc == 0), stop=(kc == 3),
                                )
                            # contiguous evacuation; alternate engines
                            dst = qkT_new[m][:]
                            src = pq[:]
                            if m % 2 == 0:
                                nc.vector.tensor_copy(dst, src)
                            else:
                                nc.scalar.activation(
                                    dst, src, mybir.ActivationFunctionType.Copy,
                                )

                if att:
                    p2_of[g0] = p2_l
                if g1 + 1 < NGRUN:
                    while len(xp_next) < 4:
                        xp_next.append(pad_copy(g1 + 1, len(xp_next)))
                    xp_of[g1 + 1] = xp_next

                # tail iterations have no GEMM stage: run proj early so it
                # covers the last group's softmax-chain latency
                if g1 >= NGRUN and 0 <= gp < NGRUN:
                    emit_proj(gp)

                # ---------- v GEMM (g1) interleaved with po bundles (g0) ------
                if att:
                    oT = otpool.tile([128, 4, NB], F16, tag="oT", name="oT")
                    oT_of[g0] = oT
                for sub in range(4):
                    if att:
                        v_sb0 = vsb_of[g0][sub]
                        p2 = p2_of[g0][sub]
                        for mp in range(4):
                            for e in range(2):
                                for b4 in range(4):
                                    nc.tensor.matmul(
                                        po[b4][64 * e:64 * e + 64, sub, mp, :],
                                        v_sb0[32 * b4:32 * b4 + VV,
                                              128 * mp + 64 * e:128 * mp + 64 * e + 64],
                                        p2[32 * b4:32 * b4 + VV, mp, e, :],
                                        start=True, stop=True,
                                        tile_position=(32 * b4, 64 * e),
                                    )
                        # evacuate po -> oT slots (split ACT/DVE)
                        oT0 = oT_of[g0]
                        oT0v = oT0[:].rearrange("p m (s v) -> p m s v", v=VV)
                        for b4 in range(4):
                            if b4 % 2 == 0:
                                nc.vector.tensor_copy(
                                    oT0v[:, :, 4 * sub + b4, :], po[b4][:, sub, :, :],
                                )
                            else:
                                nc.scalar.activation(
                                    oT0v[:, :, 4 * sub + b4, :], po[b4][:, sub, :, :],
                                    mybir.ActivationFunctionType.Copy,
                                )

                    if g1 < NGRUN:
                        xp = xp_of[g1]
                        pv = pbig.tile([128, C], F32, tag="big", name="pv")
                        for kc in range(4):
                            nc.tensor.matmul(
                                pv[:],
                                xp[kc][:, 4 * sub:4 * sub + 4, :],
                                wv_r[kc][:],
                                start=(kc == 0), stop=(kc == 3),
                            )
                        v_sb = vpool.tile([128, C], F16, tag=f"v{sub}", name=f"v{sub}")
                        nc.vector.tensor_copy(v_sb[:], pv[:])
                        vsb_of.setdefault(g1, []).append(v_sb)

                # ---------- proj (gp) ----------------------------------------
                if g1 < NGRUN and 0 <= gp < NGRUN:
                    emit_proj(gp)

    return nc


LAST_RESULT = {}


def kernel(x: np.ndarray, w_qkv: np.ndarray, w_proj: np.ndarray,
           _trace: bool = False) -> np.ndarray:
    n, c, t, vv = x.shape
    assert (n, c, t, vv) == (16, 512, 256, 25)
    scale = np.float32((c // H) ** -0.5)

    wq = w_qkv[:c] * scale
    wk = w_qkv[c:2 * c]
    wv = w_qkv[2 * c:]
    wqkT = np.ascontiguousarray(np.concatenate([wq, wk], axis=0).T.astype(np.float16))
    wvT = np.ascontiguousarray(wv.T.astype(np.float16))
    wprojT = np.ascontiguousarray(w_proj.T.astype(np.float16))

    nc = build_nc()
    split_excess_waits(nc)
    in_maps = []
    for core in range(N_CORES):
        shard = np.ascontiguousarray(
            x[core * NN_PER_CORE:(core + 1) * NN_PER_CORE].astype(np.float16)
        )
        in_maps.append({"x": shard, "wqkT": wqkT, "wvT": wvT, "wprojT": wprojT})

    kw = {}
    if _trace:
        import tempfile
        kw = dict(trace=True, tmpdir=tempfile.mkdtemp(prefix="attn2_trace_"))
    res = run_bass_kernel_spmd(nc, in_maps, list(range(N_CORES)), **kw)
    LAST_RESULT["res"] = res
    LAST_RESULT["tmpdir"] = kw.get("tmpdir")
    out = np.empty((n, c, t, vv), dtype=np.float32)
    for core in range(N_CORES):
        out[core * NN_PER_CORE:(core + 1) * NN_PER_CORE] = \
            res.results[core]["y"].astype(np.float32)
    return out
